# revision 8
# baseline (speedup 1.0000x reference)
"""Trainium2 Bass kernel for 2-layer GAT + graph pooling + MLP.

Sharding: nodes dst-sharded across 8 cores (6250 contiguous nodes each).
Each core replicates the node transform into a per-core-ROTATED fat table in
HBM (row r = node (core_base + r) % N) so its own dst range sits at local
rows [0, 6250). Fat row (bf16, 768B gather elem, 536B written):
  [ h0(64) 1 h1(64) 1 h2(64) 1 h3(64) 1 | esrc 4 x f32 | unused ]
The interleaved per-head 1-columns let one batched multiply produce messages
AND softmax-denominator lanes in a single segment-sum matmul operand.

Edge phase per 128-dst-node window: dma_gather src fat rows (2 int16-safe
half views, trailing -1 indices skip padding), batched onehot generation via
stride-0 broadcast APs, per-edge dst logits via a transposed-onehot matmul
against a resident edst table (no per-edge dst DMA gather), one batched
logit chain + message scale, selector-matmul segment-sum.

3 launches: L1, L2 (+pool partials), L3 (MLP head). Host work is layout
only: transposes, rotations, edge sorting, padding, weight repacking.
"""
import numpy as np
import ml_dtypes

import concourse.bass as bass
import concourse.bacc as bacc
import concourse.tile as tile
from concourse import mybir
from concourse.bass_utils import run_bass_kernel_spmd

BF16 = mybir.dt.bfloat16
F32 = mybir.dt.float32
I16 = mybir.dt.int16
P = 128
NCORES = 8
ROW = 384          # fat table row stride, bf16 elems (768B gather elem)
RWR = 268          # written row prefix: 260 interleaved + 8 esrc slots
HALF = 32768       # int16-safe table half
TRASH = 3000.0     # c-value for pad edges -> all-zero onehot column
REPC = 4           # ohT generation chunk (tiles per PSUM-bank bcast)
CH = 8             # phase A node tiles per DMA chunk


def cfg_full():
    return dict(N=50000, E=800000, IN_CH=128, HID=64, HEADS=4, G=64)


def _wrap_idx(idx):
    """[n] int -> [128, n//16] int16 gather layout (16-wrap, replicated 8x)."""
    n = idx.shape[0]
    assert n % 16 == 0
    return np.tile(idx.reshape(n // 16, 16).T, (8, 1)).astype(np.int16)


def _flat_att(a):
    """[H, D] attention vec -> [H*D, H] block matrix A with A[h*D+d, h]=a[h,d]."""
    H, D = a.shape
    A = np.zeros((H * D, H), np.float32)
    for h in range(H):
        A[h * D:(h + 1) * D, h] = a[h]
    return A


class EdgePlan:
    """Host-side per-core edge layout for one graph (shared by both layers)."""

    def __init__(self, src, dst, N, core):
        base = core * (N // NCORES)
        nloc = N // NCORES
        self.n_win = (nloc + P - 1) // P
        m = (dst >= base) & (dst < base + nloc)
        s, d = src[m], dst[m] - base
        order = np.argsort(d, kind="stable")
        s, d = s[order], d[order]
        rot = (s - base) % N          # rotated src row
        self.win_edges = []           # per window: (rotA, rotB, clocA, clocB)
        for w in range(self.n_win):
            lo, hi = np.searchsorted(d, [w * P, (w + 1) * P])
            rs, dl = rot[lo:hi], d[lo:hi] - w * P
            h1 = rs < HALF
            self.win_edges.append((rs[h1], rs[~h1] - HALF, dl[h1], dl[~h1]))

    @staticmethod
    def tile_counts(plans):
        n_win = plans[0].n_win
        T1, T2 = [], []
        for w in range(n_win):
            m1 = max(len(p.win_edges[w][0]) for p in plans)
            m2 = max(len(p.win_edges[w][1]) for p in plans)
            T1.append(max(1, -(-m1 // P)))
            T2.append(max(1, -(-m2 // P)))
        return T1, T2

    def arrays(self, T1, T2, maxtt):
        """fatA, fatB (idx, -1 pads), cvt [128, ntiles], cvrow [n_win, maxtt*P]."""
        fatA, fatB, cvt, cvr = [], [], [], []
        for w in range(self.n_win):
            rs1, rs2, c1, c2 = self.win_edges[w]
            n1, n2 = T1[w] * P, T2[w] * P
            a = np.zeros(n1, np.int64); a[:len(rs1)] = rs1
            b = np.zeros(n2, np.int64); b[:len(rs2)] = rs2
            c = np.full(n1 + n2, TRASH, np.float32)
            c[:len(c1)] = c1
            c[n1:n1 + len(c2)] = c2
            fatA.append(_wrap_idx(a)); fatB.append(_wrap_idx(b))
            cvt.append(c.reshape(-1, P).T)
            row = np.full(maxtt * P, TRASH, np.float32)
            row[:n1 + n2] = c
            cvr.append(row)
        bf = ml_dtypes.bfloat16
        return (np.concatenate(fatA, 1), np.concatenate(fatB, 1),
                np.concatenate(cvt, 1).astype(bf), np.stack(cvr).astype(bf))


def build_layer(C, n_nodes_pad, T1, T2, layer, pool_meta=False):
    K = C["IN_CH"] if layer == 1 else C["HEADS"] * C["HID"]
    HO = C["HEADS"] * C["HID"]          # 256
    H = C["HEADS"]; HID = C["HID"]      # 4, 64
    BL = HID + 1                        # interleaved block: 64 msg + 1 one
    SEGW = H * BL                       # 260
    nloc = C["N"] // NCORES
    n_win = (nloc + P - 1) // P
    nloc_pad = n_win * P
    ntiles = sum(T1) + sum(T2)
    maxtt = max(t1 + t2 for t1, t2 in zip(T1, T2))
    NROWS = n_nodes_pad
    G = C["G"]
    IN = C["IN_CH"]
    kh = K // P
    EXTW = HO + 2 * H                   # 264 rhs cols per k-block

    nc = bacc.Bacc("TRN2", debug=False, num_devices=NCORES,
                   num_swdge_queues=4, dynamic_dma_scratch_size=16384)

    xT_res = nc.dram_tensor("xT_res", [P, NROWS], BF16, kind="ExternalInput")
    if kh == 2:
        xT_str = nc.dram_tensor("xT_str", [P, NROWS], BF16, kind="ExternalInput")
    rhs_d = nc.dram_tensor("rhs_ext", [P, kh * EXTW], BF16, kind="ExternalInput")
    bias_d = nc.dram_tensor("bias_bc", [P, HO], BF16, kind="ExternalInput")
    iota_r_d = nc.dram_tensor("iota_r", [P, P], BF16, kind="ExternalInput")
    iota_c_d = nc.dram_tensor("iota_c", [P, 1], BF16, kind="ExternalInput")
    ones_d = nc.dram_tensor("ones1", [1, P], BF16, kind="ExternalInput")
    cvbf_d = nc.dram_tensor("cvbf", [P, ntiles], BF16, kind="ExternalInput")
    cvrow_d = nc.dram_tensor("cvrow", [n_win, maxtt * P], BF16, kind="ExternalInput")
    fatA_d = nc.dram_tensor("fatA", [P, sum(T1) * 8], I16, kind="ExternalInput")
    fatB_d = nc.dram_tensor("fatB", [P, sum(T2) * 8], I16, kind="ExternalInput")
    table = nc.dram_tensor("table", [NROWS * ROW], BF16, kind="Internal")
    if pool_meta:
        x_own = nc.dram_tensor("x_own", [nloc_pad, IN], BF16, kind="ExternalInput")
        h1_own = nc.dram_tensor("h1_own", [nloc_pad, HO], BF16, kind="ExternalInput")
        g_ids = nc.dram_tensor("g_ids", [P, n_win], F32, kind="ExternalInput")
        poolT = nc.dram_tensor("poolT", [G, IN + 2 * HO], F32, kind="ExternalOutput")
    else:
        h_own = nc.dram_tensor("h_own", [nloc_pad, HO], BF16, kind="ExternalOutput")

    viewA = bass.AP(table, 0, [[ROW, min(HALF, NROWS)], [1, ROW]])
    nB = max(1, NROWS - HALF)
    viewB = bass.AP(table, HALF * ROW, [[ROW, nB], [1, ROW]])
    ntile_nodes = NROWS // P

    with tile.TileContext(nc) as tc:
        ctx_pools = []
        _cm = tc.tile_pool(name="res", bufs=1); resP = _cm.__enter__(); ctx_pools.append(_cm)
        # ---------- resident tensors ----------
        rhs_sb = resP.tile([P, kh * EXTW], BF16)
        nc.sync.dma_start(rhs_sb[:], rhs_d[:, :])
        bias_sb = resP.tile([P, HO], BF16)
        nc.sync.dma_start(bias_sb[:], bias_d[:, :])
        iota_r = resP.tile([P, P], BF16)
        nc.sync.dma_start(iota_r[:], iota_r_d[:, :])
        iota_c = resP.tile([P, 1], BF16)
        nc.sync.dma_start(iota_c[:], iota_c_d[:, :])
        ones_sb = resP.tile([1, P], BF16)
        nc.sync.dma_start(ones_sb[:], ones_d[:, :])
        cv_sb = resP.tile([P, ntiles], BF16)
        nc.sync.dma_start(cv_sb[:], cvbf_d[:, :])
        edst_all = resP.tile([P, n_win * H], BF16)
        _cm2 = tc.tile_pool(name="idx", bufs=1); idxP = _cm2.__enter__(); ctx_pools.append(_cm2)
        fatA_sb = idxP.tile([P, sum(T1) * 8], I16)
        nc.sync.dma_start(fatA_sb[:], fatA_d[:, :])
        fatB_sb = idxP.tile([P, sum(T2) * 8], I16)
        nc.sync.dma_start(fatB_sb[:], fatB_d[:, :])
        if pool_meta:
            gid_sb = resP.tile([P, n_win], F32)
            nc.sync.dma_start(gid_sb[:], g_ids[:, :])

        # ---------- phase A: node transform -> fat table ----------
        with tc.tile_pool(name="pa_ps", bufs=2, space="PSUM") as pa_ps, \
             tc.tile_pool(name="pa_sb", bufs=3) as pa_sb, \
             tc.tile_pool(name="pa_w", bufs=2) as pa_w:
            for ch in range(0, ntile_nodes, CH):
                nt = min(CH, ntile_nodes - ch)
                xa = pa_w.tile([P, CH * P], BF16, tag="xa")
                nc.sync.dma_start(xa[:, :nt * P], xT_res[:, ch * P:(ch + nt) * P])
                if kh == 2:
                    xb = pa_w.tile([P, CH * P], BF16, tag="xb")
                    nc.sync.dma_start(xb[:, :nt * P], xT_str[:, ch * P:(ch + nt) * P])
                extc = pa_sb.tile([P, CH * RWR], BF16, tag="extc")
                ea = extc[:]
                if ch < 3 * CH:  # ones-columns survive buffer rotation
                    nc.vector.memset(
                        bass.AP(ea.tensor, ea.offset + HID,
                                [ea.ap[0], [RWR, CH], [BL, H], [1, 1]]), 1.0)
                for j in range(nt):
                    t = ch + j
                    ps = pa_ps.tile([P, EXTW], F32, tag="pa")
                    nc.tensor.matmul(ps[:], xa[:, j * P:(j + 1) * P],
                                     rhs_sb[:, 0:EXTW], start=True, stop=(kh == 1))
                    if kh == 2:
                        nc.tensor.matmul(ps[:], xb[:, j * P:(j + 1) * P],
                                         rhs_sb[:, EXTW:2 * EXTW],
                                         start=False, stop=True)
                    nc.scalar.copy(
                        bass.AP(ea.tensor, ea.offset + j * RWR,
                                [ea.ap[0], [BL, H], [1, HID]]),
                        bass.AP(ps[:].tensor, ps[:].offset,
                                [ps[:].ap[0], [HID, H], [1, HID]]))
                    nc.vector.tensor_copy(
                        extc[:, j * RWR + H * BL:j * RWR + H * BL + 2 * H].bitcast(F32),
                        ps[:, HO:HO + H])
                    if t < n_win:
                        nc.vector.tensor_copy(edst_all[:, t * H:(t + 1) * H],
                                              ps[:, HO + H:HO + 2 * H])
                nc.sync.dma_start(
                    bass.AP(table, ch * P * ROW,
                            [[ROW, P], [P * ROW, nt], [1, RWR]]),
                    bass.AP(ea.tensor, ea.offset,
                            [ea.ap[0], [RWR, nt], [1, RWR]]))

        # ---------- edge phase ----------
        GB = 3
        with tc.tile_pool(name="eg_rep", bufs=2, space="PSUM") as eg_rep, \
             tc.tile_pool(name="eg_ed", bufs=2, space="PSUM") as eg_ed, \
             tc.tile_pool(name="eg_ps", bufs=2, space="PSUM") as eg_ps, \
             (tc.tile_pool(name="pool_ps", bufs=1, space="PSUM") if pool_meta
              else tc.tile_pool(name="dummy", bufs=1)) as pl_ps, \
             tc.tile_pool(name="eg_g", bufs=GB) as eg_g, \
             tc.tile_pool(name="eg_oh", bufs=2) as eg_oh, \
             tc.tile_pool(name="eg_m", bufs=2) as eg_m, \
             tc.tile_pool(name="eg_s", bufs=2) as eg_s, \
             tc.tile_pool(name="eg_row", bufs=2) as eg_row:
            if pool_meta:
                DC = IN + 2 * HO                      # 640
                poolA = pl_ps.tile([G, 512], F32, name="poolA")
                poolB = pl_ps.tile([G, DC - 512], F32, name="poolB")
            colA = colB = cvoff = 0
            for w in range(n_win):
                t1, t2 = T1[w], T2[w]
                tt = t1 + t2
                # cvrow staging
                cvw = eg_row.tile([1, maxtt * P], BF16, tag="cvw")
                nc.sync.dma_start(cvw[:, :tt * P], cvrow_d[w:w + 1, :tt * P])
                # gathers (trailing -1 indices are skipped)
                gbuf = eg_g.tile([P, maxtt * ROW], BF16, tag="gbuf")
                if w < GB:
                    nc.vector.memset(gbuf[:], 0.0)
                nc.gpsimd.dma_gather(
                    out_ap=gbuf[:, 0:t1 * ROW].rearrange("p (t e) -> p t e", e=ROW),
                    in_ap=viewA, idxs_ap=fatA_sb[:, colA:colA + t1 * 8],
                    num_idxs=t1 * P, num_idxs_reg=t1 * P,
                    elem_size=ROW, elem_step=ROW,
                    queue_num=w % 2, single_packet=False)
                nc.gpsimd.dma_gather(
                    out_ap=gbuf[:, t1 * ROW:tt * ROW].rearrange("p (t e) -> p t e", e=ROW),
                    in_ap=viewB, idxs_ap=fatB_sb[:, colB:colB + t2 * 8],
                    num_idxs=t2 * P, num_idxs_reg=t2 * P,
                    elem_size=ROW, elem_step=ROW,
                    queue_num=2 + w % 2, single_packet=False)
                colA += t1 * 8; colB += t2 * 8

                # onehot oh[p=e, (t,d)] = (cv[p,t] == d)
                oh = eg_oh.tile([P, maxtt * P], BF16, tag="oh")
                cva = cv_sb[:, cvoff:cvoff + tt]
                nc.vector.tensor_tensor(
                    out=oh[:, :tt * P],
                    in0=bass.AP(cva.tensor, cva.offset,
                                [cva.ap[0], [1, tt], [0, P]]),
                    in1=bass.AP(iota_r[:].tensor, iota_r[:].offset,
                                [iota_r[:].ap[0], [0, tt], [1, P]]),
                    op=mybir.AluOpType.is_equal)
                # transposed onehot ohT[p=d, (t,e)] = (p == dl(t,e))
                ohT = eg_oh.tile([P, maxtt * P], BF16, tag="ohT")
                for c0 in range(0, tt, REPC):
                    cc = min(REPC, tt - c0)
                    rep = eg_rep.tile([P, REPC * P], F32, tag="rep")
                    nc.tensor.matmul(rep[:, :cc * P], ones_sb[:],
                                     cvw[:, c0 * P:(c0 + cc) * P],
                                     start=True, stop=True)
                    nc.vector.tensor_tensor(
                        out=ohT[:, c0 * P:(c0 + cc) * P],
                        in0=bass.AP(iota_c[:].tensor, iota_c[:].offset,
                                    [iota_c[:].ap[0], [0, cc], [0, P]]),
                        in1=rep[:, :cc * P],
                        op=mybir.AluOpType.is_equal)
                # per-edge dst logits: edst[e, (t,h)] = sum_d ohT[d,(t,e)]*edst_all[d,(w,h)]
                edps = eg_ed.tile([P, maxtt * H], F32, tag="edps")
                for t in range(tt):
                    nc.tensor.matmul(edps[:, t * H:(t + 1) * H],
                                     ohT[:, t * P:(t + 1) * P],
                                     edst_all[:, w * H:(w + 1) * H],
                                     start=True, stop=True)
                # logit chain: ex = exp(lrelu(esrc + edst))
                gap = gbuf[:].bitcast(F32)
                esrc_ap = bass.AP(gap.tensor, gap.offset + H * BL // 2,
                                  [gap.ap[0], [ROW // 2, tt], [1, H]])
                logit = eg_s.tile([P, maxtt * H], F32, tag="logit")
                nc.vector.tensor_tensor(out=logit[:, :tt * H], in0=esrc_ap,
                                        in1=edps[:, :tt * H],
                                        op=mybir.AluOpType.add)
                lr = eg_s.tile([P, maxtt * H], F32, tag="lr")
                nc.vector.scalar_tensor_tensor(
                    out=lr[:, :tt * H], in0=logit[:, :tt * H], scalar=0.2,
                    in1=logit[:, :tt * H],
                    op0=mybir.AluOpType.mult, op1=mybir.AluOpType.max)
                exq = eg_s.tile([P, maxtt * H], BF16, tag="exq")
                nc.scalar.activation(exq[:, :tt * H], lr[:, :tt * H],
                                     mybir.ActivationFunctionType.Exp)
                # batched message scale into interleaved mbuf (ex lane = ones*ex)
                mbuf = eg_m.tile([P, maxtt * SEGW], BF16, tag="mbuf")
                ga = gbuf[:]; ma = mbuf[:]; xa_ = exq[:]
                nc.vector.tensor_tensor(
                    out=bass.AP(ma.tensor, ma.offset,
                                [ma.ap[0], [SEGW, tt], [BL, H], [1, BL]]),
                    in0=bass.AP(ga.tensor, ga.offset,
                                [ga.ap[0], [ROW, tt], [BL, H], [1, BL]]),
                    in1=bass.AP(xa_.tensor, xa_.offset,
                                [xa_.ap[0], [H, tt], [1, H], [0, BL]]),
                    op=mybir.AluOpType.mult)
                # segment-sum matmuls
                seg = eg_ps.tile([P, SEGW], F32, tag="seg")
                for t in range(tt):
                    nc.tensor.matmul(seg[:], oh[:, t * P:(t + 1) * P],
                                     mbuf[:, t * SEGW:(t + 1) * SEGW],
                                     start=(t == 0), stop=(t == tt - 1))
                cvoff += tt
                # extract h = relu(num/den + bias)
                sa = seg[:]
                den = eg_s.tile([P, H], F32, tag="den")
                nc.vector.tensor_scalar(
                    out=den[:],
                    in0=bass.AP(sa.tensor, sa.offset + HID, [sa.ap[0], [BL, H], [1, 1]]),
                    scalar1=1e-30, scalar2=None, op0=mybir.AluOpType.add)
                rec = eg_s.tile([P, H], F32, tag="rec")
                nc.vector.reciprocal(rec[:], den[:])
                hw = eg_s.tile([P, HO], BF16, tag="hw")
                ra = rec[:]
                nc.vector.tensor_tensor(
                    out=hw[:],
                    in0=bass.AP(sa.tensor, sa.offset, [sa.ap[0], [BL, H], [1, HID]]),
                    in1=bass.AP(ra.tensor, ra.offset, [ra.ap[0], [1, H], [0, HID]]),
                    op=mybir.AluOpType.mult)
                hb = eg_s.tile([P, HO], BF16, tag="hb")
                nc.vector.tensor_tensor(out=hb[:], in0=hw[:], in1=bias_sb[:],
                                        op=mybir.AluOpType.add)
                if pool_meta:
                    comb = eg_m.tile([P, DC], BF16, tag="comb")
                    nc.sync.dma_start(comb[:, 0:IN], x_own[w * P:(w + 1) * P, :])
                    nc.sync.dma_start(comb[:, IN:IN + HO],
                                      h1_own[w * P:(w + 1) * P, :])
                    nc.vector.tensor_scalar(
                        out=comb[:, IN + HO:DC], in0=hb[:], scalar1=0.0,
                        scalar2=None, op0=mybir.AluOpType.max)
                    goh = eg_s.tile([P, G], BF16, tag="goh")
                    nc.vector.tensor_scalar(
                        out=goh[:], in0=iota_r[:, 0:G],
                        scalar1=gid_sb[:, w:w + 1], scalar2=None,
                        op0=mybir.AluOpType.is_equal)
                    st = (w == 0); sp = (w == n_win - 1)
                    nc.tensor.matmul(poolA[:], goh[:], comb[:, 0:512],
                                     start=st, stop=sp)
                    nc.tensor.matmul(poolB[:], goh[:], comb[:, 512:DC],
                                     start=st, stop=sp)
                else:
                    hro = eg_s.tile([P, HO], BF16, tag="hro")
                    nc.vector.tensor_scalar(
                        out=hro[:], in0=hb[:], scalar1=0.0,
                        scalar2=None, op0=mybir.AluOpType.max)
                    nc.sync.dma_start(h_own[w * P:(w + 1) * P, :], hro[:])
            if pool_meta:
                po = eg_m.tile([G, DC], F32, tag="po")
                nc.vector.tensor_copy(po[:, 0:512], poolA[:])
                nc.vector.tensor_copy(po[:, 512:DC], poolB[:])
                nc.sync.dma_start(poolT[:, :], po[:])
        for p in reversed(ctx_pools):
            p.__exit__(None, None, None)
    nc.compile()
    return nc


def build_l3(C):
    """MLP head on 1 core: pooled mean -> relu(pooled@W3+b3)@W4+b4, transposed."""
    G = C["G"]; DC = C["IN_CH"] + 2 * C["HEADS"] * C["HID"]  # 640
    K5 = DC // P                                             # 5
    nc = bacc.Bacc("TRN2", debug=False, num_devices=1)
    parts = nc.dram_tensor("parts", [NCORES * K5 * P, G], F32, kind="ExternalInput")
    crec = nc.dram_tensor("crec", [1, G], F32, kind="ExternalInput")
    ones1 = nc.dram_tensor("ones1", [1, P], F32, kind="ExternalInput")
    W3_d = nc.dram_tensor("W3", [DC, 256], F32, kind="ExternalInput")
    b3_d = nc.dram_tensor("b3", [256, 1], F32, kind="ExternalInput")
    W4_d = nc.dram_tensor("W4", [256, P], F32, kind="ExternalInput")
    b4_d = nc.dram_tensor("b4", [P, 1], F32, kind="ExternalInput")
    out = nc.dram_tensor("out", [P, G], F32, kind="ExternalOutput")

    with tile.TileContext(nc) as tc:
        with tc.tile_pool(name="sb", bufs=1) as sb, \
             tc.tile_pool(name="ps", bufs=1, space="PSUM") as psp, \
             tc.tile_pool(name="w", bufs=2) as wp:
            ones_sb = sb.tile([1, P], F32)
            nc.sync.dma_start(ones_sb[:], ones1[:, :])
            crec_sb = sb.tile([1, G], F32)
            nc.sync.dma_start(crec_sb[:], crec[:, :])
            rb_ps = psp.tile([P, G], F32, tag="rb")
            nc.tensor.matmul(rb_ps[:], ones_sb[:], crec_sb[:], start=True, stop=True)
            rec_bc = sb.tile([P, G], F32)
            nc.vector.tensor_copy(rec_bc[:], rb_ps[:])

            pooled = [sb.tile([P, G], F32, tag=f"pl{i}", name=f"pl{i}") for i in range(K5)]
            for i in range(K5):
                acc = pooled[i]
                pt = wp.tile([P, G], F32, tag="pt")
                nc.sync.dma_start(pt[:], parts[i * P:(i + 1) * P, :])
                nc.vector.tensor_copy(acc[:], pt[:])
                for c in range(1, NCORES):
                    pt2 = wp.tile([P, G], F32, tag="pt")
                    nc.sync.dma_start(pt2[:], parts[(c * K5 + i) * P:(c * K5 + i + 1) * P, :])
                    nc.vector.tensor_tensor(out=acc[:], in0=acc[:], in1=pt2[:],
                                            op=mybir.AluOpType.add)
                nc.vector.tensor_tensor(out=acc[:], in0=acc[:], in1=rec_bc[:],
                                        op=mybir.AluOpType.mult)

            hm = [sb.tile([P, G], F32, tag=f"hm{j}", name=f"hm{j}") for j in range(2)]
            for j in range(2):
                ps1 = psp.tile([P, G], F32, tag=f"mm{j}")
                for i in range(K5):
                    wt = wp.tile([P, P], F32, tag="w3")
                    nc.sync.dma_start(wt[:], W3_d[i * P:(i + 1) * P, j * P:(j + 1) * P])
                    wt2 = wp.tile([P, P], F32, tag="w3b")
                    nc.vector.tensor_copy(wt2[:], wt[:])
                    nc.tensor.matmul(ps1[:], wt2[:], pooled[i][:],
                                     start=(i == 0), stop=(i == K5 - 1))
                bt = wp.tile([P, 1], F32, tag="b3")
                nc.sync.dma_start(bt[:], b3_d[j * P:(j + 1) * P, :])
                nc.scalar.activation(hm[j][:], ps1[:],
                                     mybir.ActivationFunctionType.Relu,
                                     bias=bt[:, 0:1], scale=1.0)
            ps2 = psp.tile([P, G], F32, tag="mm2")
            for j in range(2):
                wt = wp.tile([P, P], F32, tag="w4")
                nc.sync.dma_start(wt[:], W4_d[j * P:(j + 1) * P, :])
                wt2 = wp.tile([P, P], F32, tag="w4b")
                nc.vector.tensor_copy(wt2[:], wt[:])
                nc.tensor.matmul(ps2[:], wt2[:], hm[j][:],
                                 start=(j == 0), stop=(j == 1))
            bt4 = wp.tile([P, 1], F32, tag="b4")
            nc.sync.dma_start(bt4[:], b4_d[:, :])
            ot = sb.tile([P, G], F32)
            nc.scalar.activation(ot[:], ps2[:],
                                 mybir.ActivationFunctionType.Identity,
                                 bias=bt4[:, 0:1], scale=1.0)
            nc.sync.dma_start(out[:, :], ot[:])
    nc.compile()
    return nc


def _prep_rot_xT(x, base, n_pad):
    """x [N, K] f32 -> rotated xT [K, n_pad] bf16."""
    N = x.shape[0]
    xr = np.zeros((n_pad, x.shape[1]), np.float32)
    idx = (base + np.arange(N)) % N
    xr[:N] = x[idx]
    return np.ascontiguousarray(xr.T).astype(ml_dtypes.bfloat16)


def _rhs_ext(W, a_src, a_dst):
    """Host weight repack: [P, kh*264] = per k-block [W | W@A_src | W@A_dst]."""
    W = np.asarray(W, np.float32)
    K, HO = W.shape
    Ws = W @ _flat_att(np.asarray(a_src, np.float32))
    Wd = W @ _flat_att(np.asarray(a_dst, np.float32))
    kh = K // P
    blocks = [np.concatenate([W[j * P:(j + 1) * P],
                              Ws[j * P:(j + 1) * P],
                              Wd[j * P:(j + 1) * P]], 1) for j in range(kh)]
    return np.concatenate(blocks, 1).astype(ml_dtypes.bfloat16)


def kernel(x, edge_index, batch, W1, a1_src, a1_dst, b1, W2, a2_src, a2_dst, b2,
           W3, b3, W4, b4, _trace=False, _timings=None):
    C = cfg_full()
    N, E, G = C["N"], C["E"], C["G"]
    IN, HO, H = C["IN_CH"], C["HEADS"] * C["HID"], C["HEADS"]
    nloc = N // NCORES
    n_win = (nloc + P - 1) // P
    nloc_pad = n_win * P

    x = np.asarray(x, np.float32)
    src = np.asarray(edge_index[0], np.int64)
    dst = np.asarray(edge_index[1], np.int64)
    batch = np.asarray(batch, np.int64)
    bf = ml_dtypes.bfloat16

    ntn = -(-N // P) + 1
    NROWS = ntn * P

    plans = [EdgePlan(src, dst, N, c) for c in range(NCORES)]
    T1, T2 = EdgePlan.tile_counts(plans)
    maxtt = max(t1 + t2 for t1, t2 in zip(T1, T2))
    earr = [p.arrays(T1, T2, maxtt) for p in plans]

    iota_r = np.tile(np.arange(P, dtype=np.float32), (P, 1)).astype(bf)
    iota_c = np.arange(P, dtype=np.float32).reshape(P, 1).astype(bf)
    ones1 = np.ones((1, P), np.float32).astype(bf)
    common = dict(iota_r=iota_r, iota_c=iota_c, ones1=ones1)

    # ---------------- L1 ----------------
    nc1 = build_layer(C, NROWS, T1, T2, layer=1)
    rhs1 = _rhs_ext(W1, a1_src, a1_dst)
    bias1 = np.tile(np.asarray(b1, np.float32).reshape(1, HO), (P, 1)).astype(bf)
    in_maps = []
    for c in range(NCORES):
        fA, fB, cvt, cvr = earr[c]
        in_maps.append({
            "xT_res": _prep_rot_xT(x, c * nloc, NROWS),
            "rhs_ext": rhs1, "bias_bc": bias1,
            "cvbf": cvt, "cvrow": cvr, "fatA": fA, "fatB": fB, **common,
        })
    r1 = run_bass_kernel_spmd(nc1, in_maps, core_ids=list(range(NCORES)), trace=_trace)
    if _timings is not None and r1.exec_time_ns:
        _timings.append(("L1", r1.exec_time_ns))
    h1_own = [r1.results[c]["h_own"] for c in range(NCORES)]
    h1_full = np.concatenate([h.astype(np.float32)[:nloc] for h in h1_own])

    # ---------------- L2 ----------------
    nc2 = build_layer(C, NROWS, T1, T2, layer=2, pool_meta=True)
    rhs2 = _rhs_ext(W2, a2_src, a2_dst)
    bias2 = np.tile(np.asarray(b2, np.float32).reshape(1, HO), (P, 1)).astype(bf)
    in_maps2 = []
    for c in range(NCORES):
        fA, fB, cvt, cvr = earr[c]
        base = c * nloc
        h1T = _prep_rot_xT(h1_full, base, NROWS)
        xo = np.zeros((nloc_pad, IN), np.float32)
        xo[:nloc] = x[base:base + nloc]
        h1o = np.zeros((nloc_pad, HO), np.float32)
        h1o[:nloc] = h1_full[base:base + nloc]
        gi = np.full(nloc_pad, 3000.0, np.float32)
        gi[:nloc] = batch[base:base + nloc]
        in_maps2.append({
            "xT_res": np.ascontiguousarray(h1T[0:P]),
            "xT_str": np.ascontiguousarray(h1T[P:2 * P]),
            "rhs_ext": rhs2, "bias_bc": bias2,
            "cvbf": cvt, "cvrow": cvr, "fatA": fA, "fatB": fB, **common,
            "x_own": xo.astype(bf), "h1_own": h1o.astype(bf),
            "g_ids": np.ascontiguousarray(gi.reshape(n_win, P).T),
        })
    r2 = run_bass_kernel_spmd(nc2, in_maps2, core_ids=list(range(NCORES)), trace=_trace)
    if _timings is not None and r2.exec_time_ns:
        _timings.append(("L2", r2.exec_time_ns))

    # ---------------- L3 ----------------
    # poolT [G, 640] per core -> transpose to dc-major [5*128, G] rows per core
    parts = np.concatenate(
        [np.ascontiguousarray(r2.results[c]["poolT"].T) for c in range(NCORES)], 0)
    cnt = np.bincount(batch, minlength=G).astype(np.float32)
    crec = (1.0 / np.maximum(cnt, 1.0)).reshape(1, G)
    nc3 = build_l3(C)
    in3 = {
        "parts": parts.astype(np.float32), "crec": crec,
        "ones1": np.ones((1, P), np.float32),
        "W3": np.asarray(W3, np.float32), "b3": np.asarray(b3, np.float32).reshape(256, 1),
        "W4": np.asarray(W4, np.float32), "b4": np.asarray(b4, np.float32).reshape(P, 1),
    }
    r3 = run_bass_kernel_spmd(nc3, [in3], core_ids=[0], trace=_trace)
    if _timings is not None and r3.exec_time_ns:
        _timings.append(("L3", r3.exec_time_ns))
    return np.ascontiguousarray(r3.results[0]["out"].T.astype(np.float32))
